# revision 1
# baseline (speedup 1.0000x reference)
"""Trainium2 Bass kernel for nn_AudioVideoInter (ragged_sequence).

Semantics (see reference): for each batch b,
  lab   = (labels[b] == 1)                       selection mask over T frames
  mean  = mean_c(video[:, b, :])                 per-frame channel mean  [T]
  vm    = compacted mean[lab]                    t selected means, in order
  scale[p] = prod_{m = max(0,p-T+t) .. min(p,t-1)} vm[m]
  out[:, b, :] = audio[:, b, :] * scale[:, None]

Closed form used on-device (with cq = forward cumprod over T of
w = (lab ? mean : 1), cr = backward cumprod of w, P = cq[T-1],
rank = exclusive cumsum of lab, t = sum(lab)):
  scale[p] = P                          for p in [t-1, T-t]
  scale[r] = cq[j_r]                    for selected j_r with rank r <= t-2
  scale[T-t+1+r] = P / cq[j_r] = cr[j_r + 1]     (same j_r)
Implemented as one gpsimd local_scatter of (value - P) into zeros, then +P.
Valid whenever t <= 129 (t here is ~9..26, T=1024): the scattered
corrections then live entirely in the first/last 128-frame tiles, and all
middle output tiles use the plain global product P.

Sharding: pure data parallelism over batch. 8 cores x 4 batches each.
Within a core the 4 batches live at partitions {0,16,32,48}, so the per-batch
pipeline spreads over 4 of the 8 gpsimd Q7 cores and psum transposes stay on
quadrant-aligned partitions.

Structure (per core):
  phase 1: video (and, slot-gated behind it, audio) streams in; per 128-frame
    tile the channel sums go to DVE tensor_reduce / ACT activation-accumulate
    (alternating), get transposed to [b, T] via a PE matmul against a 1/C-
    scaled identity, and extend the forward cumprod cq incrementally (scan
    with carried initial).  The labels-only index pipeline runs concurrently.
  phase 2: as soon as cq completes, P is broadcast to [128, 4] via two tiny
    PE matmuls and the SIX MIDDLE output tiles start multiplying/streaming
    out immediately -- only the first/last output tiles wait for the serial
    tail (backward cumprod, fp16 scatter data, one local_scatter, +P, two
    PE transposes).
  phase 3: audio tiles x per-partition scale (split DVE tensor_scalar / ACT
    activation-scale), streamed out by DMA.
"""

import os
import numpy as np

T, B, C = 1024, 32, 512
NCORES = 8
BL = B // NCORES          # batches per core = 4
NT = T // 128             # 8 tiles of 128 frames
SP = 16                   # partition stride between batches
PP = BL * SP              # 64 partitions used by the per-batch pipeline

_CACHE = {}
LAST_RESULT = None        # BassKernelResults of the most recent run (for test.py)


def _build_nc():
    import concourse.bass as bass
    import concourse.tile as tile
    from concourse import bacc, mybir
    from concourse.masks import make_identity

    f32 = mybir.dt.float32
    f16 = mybir.dt.float16
    i32 = mybir.dt.int32
    i16 = mybir.dt.int16
    Alu = mybir.AluOpType
    Ax = mybir.AxisListType

    nc = bacc.Bacc("TRN2", target_bir_lowering=False, debug=False)

    video = nc.dram_tensor("video_feat", [T, BL, C], f32, kind="ExternalInput").ap()
    audio = nc.dram_tensor("audio_feat", [T, BL, C], f32, kind="ExternalInput").ap()
    labels = nc.dram_tensor("labels", [BL, T], i32, kind="ExternalInput").ap()
    out = nc.dram_tensor("out", [T, BL, C], f32, kind="ExternalOutput").ap()

    ActFn = mybir.ActivationFunctionType

    with tile.TileContext(nc) as tc:
        with (
            tc.tile_pool(name="inb", bufs=12) as in_pool,
            tc.tile_pool(name="outp", bufs=4) as out_pool,
            tc.tile_pool(name="small", bufs=1) as small,
            tc.tile_pool(name="psum", bufs=2, space="PSUM") as psum,
        ):
            # ---- constants / init (gpsimd, off the DVE critical path) ----
            ident = small.tile([128, 128], f32)
            make_identity(nc, ident[:])
            # identity scaled by 1/C: the means transpose then yields means
            # (not sums) for free
            ident_m = small.tile([128, 128], f32)
            nc.gpsimd.memset(ident_m[:], 0.0)
            nc.gpsimd.affine_select(
                out=ident_m[:], in_=ident_m[:], compare_op=Alu.not_equal,
                fill=1.0 / C, base=0, pattern=[[-1, 128]], channel_multiplier=1,
            )
            ones_col = small.tile([1, 128], f32)
            nc.gpsimd.memset(ones_col[:], 1.0)
            zeros = small.tile([PP, T], f32)
            nc.gpsimd.memset(zeros[:], 0.0)
            lab_i = small.tile([PP, T], i32)
            nc.gpsimd.memset(lab_i[:], 0)
            means_all = small.tile([128, NT, PP], f32)
            nc.gpsimd.memset(means_all[:], 0.0)
            means_bT = small.tile([PP, T], f32)

            # ---- labels -> lab mask; batch b sits at partition SP*b ----
            lab_i_spread = lab_i[:].rearrange("(b s) t -> b s t", s=SP)[:, 0, :]
            nc.sync.dma_start(out=lab_i_spread, in_=labels)

            # ---- big-input DMAs. Video and audio share one pool/tag: slot
            # backpressure makes audio tile k's load wait for video tile
            # k-2's reduce, so video gets the DMA bandwidth first. ----
            vts = []
            for t in range(NT):
                vt = in_pool.tile([128, BL, C], f32, tag="inb")
                nc.sync.dma_start(out=vt[:], in_=video[t * 128 : (t + 1) * 128])
                vts.append(vt)
            ats = []
            for t in range(NT):
                at = in_pool.tile([128, BL, C], f32, tag="inb")
                nc.sync.dma_start(out=at[:], in_=audio[t * 128 : (t + 1) * 128])
                ats.append(at)

            # ---- label-only pipeline (ready before video finishes) ----
            lab_f = small.tile([PP, T], f32)
            nc.vector.tensor_copy(out=lab_f[:], in_=lab_i[:])
            # 0/1 mask as int8: usable directly as copy_predicated mask, and
            # DVE converts it to fp32 on read for the arithmetic ops
            lab = small.tile([PP, T], mybir.dt.int8)
            nc.vector.tensor_single_scalar(
                out=lab[:], in_=lab_f[:], scalar=1.0, op=Alu.is_equal
            )
            t_cnt = small.tile([PP, 1], f32)
            nc.vector.tensor_reduce(out=t_cnt[:], in_=lab[:], axis=Ax.X, op=Alu.add)
            rank_i = small.tile([PP, T], f32)
            nc.vector.tensor_tensor_scan(
                out=rank_i[:], data0=lab[:], data1=zeros[:], initial=0.0,
                op0=Alu.add, op1=Alu.add,
            )
            # all index math in the inclusive-rank domain (selected j has
            # rank_excl = rank_i - 1):  maskA = (rank_i <= t-1) & lab,
            # idxA = rank_i*maskA - 1,  idxC = (rank_i + T+1-t)*maskA - 1
            tm1 = small.tile([PP, 1], f32)
            nc.vector.tensor_single_scalar(
                out=tm1[:], in_=t_cnt[:], scalar=1.0, op=Alu.subtract
            )
            ofs1 = small.tile([PP, 1], f32)
            nc.vector.tensor_scalar(
                out=ofs1[:], in0=t_cnt[:], scalar1=-1.0, scalar2=float(T + 1),
                op0=Alu.mult, op1=Alu.add,
            )
            maskA = small.tile([PP, T], f32)
            nc.vector.scalar_tensor_tensor(
                out=maskA[:], in0=rank_i[:], scalar=tm1[:], in1=lab[:],
                op0=Alu.is_le, op1=Alu.mult,
            )
            idx_cat = small.tile([PP, 2 * T], i16)
            qa = small.tile([PP, T], f32)
            nc.vector.scalar_tensor_tensor(
                out=qa[:], in0=rank_i[:], scalar=1.0, in1=maskA[:],
                op0=Alu.mult, op1=Alu.mult,
            )
            qc = small.tile([PP, T], f32)
            nc.vector.scalar_tensor_tensor(
                out=qc[:], in0=rank_i[:], scalar=ofs1[:], in1=maskA[:],
                op0=Alu.add, op1=Alu.mult,
            )

            # ---- per-frame channel sums + transpose to [b, T], and the
            # forward cumprod built incrementally per tile so only a short
            # tail remains after the last video tile lands. ----
            # Reduces split between DVE (tensor_reduce) and ACT (activation
            # accumulate) so phase 1 keeps pace with the video DMA stream.
            dummy = small.tile([128, C], f32)
            w = small.tile([PP, T], f32)
            nc.gpsimd.memset(w[:], 1.0)
            data_cat = small.tile([PP, 2 * T], f16)
            nc.gpsimd.memset(data_cat[:, 2 * T - 1 : 2 * T], 0.0)
            cq = small.tile([PP, T], f32)
            _ctx_prio = tc.high_priority(offset=200)
            _ctx_prio.__enter__()
            for t in range(NT):
                # channel sums for this 128-frame tile, written at stride SP
                means_sp = means_all[:].rearrange(
                    "p t (b s) -> p t b s", s=SP
                )
                if t % 2 == 0:
                    nc.vector.tensor_reduce(
                        out=means_sp[:, t, :, 0], in_=vts[t][:], axis=Ax.X,
                        op=Alu.add,
                    )
                else:
                    for b in range(BL):
                        nc.scalar.activation(
                            out=dummy[:], in_=vts[t][:, b, :], func=ActFn.Copy,
                            scale=1.0, accum_out=means_sp[:, t, b, 0:1],
                        )
                psum_mt = psum.tile([PP, 128], f32)
                nc.tensor.matmul(
                    psum_mt[:], means_all[:, t, :], ident_m[:], start=True, stop=True
                )
                sl = slice(t * 128, (t + 1) * 128)
                nc.vector.tensor_copy(out=means_bT[:, sl], in_=psum_mt[:])
                # w = lab ? mean : 1  (w preset to 1)
                nc.vector.copy_predicated(
                    out=w[:, sl], mask=lab[:, sl], data=means_bT[:, sl]
                )
                init = 1.0 if t == 0 else cq[:, t * 128 - 1 : t * 128]
                nc.vector.tensor_tensor_scan(
                    out=cq[:, sl], data0=w[:, sl], data1=zeros[:, sl],
                    initial=init, op0=Alu.mult, op1=Alu.add,
                )
            _ctx_prio.__exit__(None, None, None)

            nc.scalar.activation(
                out=idx_cat[:, 0:T], in_=qa[:], func=ActFn.Copy, scale=1.0,
                bias=-1.0,
            )
            nc.scalar.activation(
                out=idx_cat[:, T : 2 * T], in_=qc[:], func=ActFn.Copy, scale=1.0,
                bias=-1.0,
            )
            P_ap = cq[:, T - 1 : T]
            # P broadcast to [128, PP]: P_row = P.T (tiny matmul), then
            # ones_col.T @ P_row.  Ready right after the last cq slice --
            # tiles 1..NT-2 of the output only need P (t <= 129 guarantees
            # the scattered corrections live in tiles 0 and NT-1).
            psum_pr = psum.tile([1, PP], f32)
            nc.tensor.matmul(
                psum_pr[:], cq[:, T - 1 : T], ident[0:PP, 0:PP],
                start=True, stop=True,
            )
            p_row = small.tile([1, PP], f32)
            nc.vector.tensor_copy(out=p_row[:], in_=psum_pr[:])
            psum_pb = psum.tile([128, PP], f32)
            nc.tensor.matmul(
                psum_pb[:], ones_col[:], p_row[:], start=True, stop=True
            )
            p_bcast = small.tile([128, PP], f32)
            nc.vector.tensor_copy(out=p_bcast[:], in_=psum_pb[:])
            # scatter data (value - P) in fp16: [A | C] in one scatter
            nc.vector.tensor_scalar(
                out=data_cat[:, 0:T], in0=cq[:], scalar1=P_ap, scalar2=None,
                op0=Alu.subtract,
            )
            # backward cumprod: cr[j] = prod_{j' >= j} w[j']   (reversed APs)
            cr = small.tile([PP, T], f32)
            nc.vector.tensor_tensor_scan(
                out=cr[:, ::-1], data0=w[:, ::-1], data1=zeros[:], initial=1.0,
                op0=Alu.mult, op1=Alu.add,
            )
            # dataC[j] = cr[j+1] - P  (j = T-1 never scattered; its data slot
            # was zeroed in the preamble)
            nc.vector.tensor_scalar(
                out=data_cat[:, T : 2 * T - 1], in0=cr[:, 1:T], scalar1=P_ap,
                scalar2=None, op0=Alu.subtract,
            )
            dst = small.tile([PP, T], f16)
            nc.gpsimd.local_scatter(
                out_ap=dst[:], data_ap=data_cat[:], idxs_ap=idx_cat[:],
                channels=PP, num_elems=T, num_idxs=2 * T,
            )
            # middle tiles EMITTED FIRST: they only wait on P, and emitting
            # them before the scatter-dependent combine/transpose ops keeps
            # the in-order DVE/ACT streams from stalling behind the scatter
            def _mult_tile(t, s_col):
                ot = out_pool.tile([128, BL, C], f32, tag="ot")
                for b in range(BL):
                    s_ap = s_col(b)
                    if b < BL // 2:
                        nc.vector.tensor_scalar_mul(
                            out=ot[:, b, :], in0=ats[t][:, b, :], scalar1=s_ap
                        )
                    else:
                        nc.scalar.mul(out=ot[:, b, :], in_=ats[t][:, b, :], mul=s_ap)
                nc.sync.dma_start(out=out[t * 128 : (t + 1) * 128], in_=ot[:])

            for t in range(1, NT - 1):
                _mult_tile(t, lambda b: p_bcast[:, SP * b : SP * b + 1])

            # scale = dst + P, but only the first/last 128 frames carry
            # scattered corrections -- transpose just those two column blocks
            scale_ends = small.tile([PP, 2, 128], f32)
            nc.vector.tensor_scalar_add(
                out=scale_ends[:, 0, :], in0=dst[:, 0:128], scalar1=P_ap
            )
            nc.vector.tensor_scalar_add(
                out=scale_ends[:, 1, :], in0=dst[:, T - 128 : T], scalar1=P_ap
            )
            scale_jb = small.tile([128, 2, PP], f32)
            for k in range(2):
                pst = psum.tile([128, PP], f32)
                nc.tensor.matmul(
                    pst[:], scale_ends[:, k, :], ident[0:PP, 0:PP],
                    start=True, stop=True,
                )
                nc.vector.tensor_copy(out=scale_jb[:, k, :], in_=pst[:])

            _mult_tile(0, lambda b: scale_jb[:, 0, SP * b : SP * b + 1])
            _mult_tile(NT - 1, lambda b: scale_jb[:, 1, SP * b : SP * b + 1])

    nc.compile()
    return nc


def _get_nc():
    if "nc" not in _CACHE:
        _CACHE["nc"] = _build_nc()
    return _CACHE["nc"]


def _ensure_ntff_hook():
    """The agent image's antenv lacks axon_hooks; provide it and register the
    ctypes-based NTFF profiling hook so trace=True works under axon."""
    import sys
    import types

    if "antenv.axon_hooks" in sys.modules:
        return
    mod = types.ModuleType("antenv.axon_hooks")
    state = {"hook": None}
    mod.set_axon_ntff_profile_hook = lambda h: state.__setitem__("hook", h)
    mod.get_axon_ntff_profile_hook = lambda: state["hook"]
    sys.modules["antenv.axon_hooks"] = mod
    try:
        from trn_agent_boot.trn_boot import _ntff_profile_via_ctypes

        so_path = "/opt/axon/libaxon_pjrt.so"
        if os.path.exists(so_path):
            mod.set_axon_ntff_profile_hook(_ntff_profile_via_ctypes(so_path))
    except Exception:
        pass


def kernel(video_feat: np.ndarray, audio_feat: np.ndarray, labels: np.ndarray) -> np.ndarray:
    global LAST_RESULT
    from concourse.bass_utils import run_bass_kernel_spmd

    video_feat = np.ascontiguousarray(video_feat, dtype=np.float32)
    audio_feat = np.ascontiguousarray(audio_feat, dtype=np.float32)
    labels = np.ascontiguousarray(labels, dtype=np.int32)

    nc = _get_nc()
    in_maps = []
    for m in range(NCORES):
        bs = slice(m * BL, (m + 1) * BL)
        in_maps.append(
            {
                "video_feat": np.ascontiguousarray(video_feat[:, bs, :]),
                "audio_feat": np.ascontiguousarray(audio_feat[:, bs, :]),
                "labels": np.ascontiguousarray(labels[bs, :]),
            }
        )

    trace = bool(os.environ.get("KERNEL_PROFILE"))
    if trace:
        _ensure_ntff_hook()
    kwargs = {}
    if trace and os.environ.get("KERNEL_PROFILE_ALL_CORES"):
        kwargs["trace_cores"] = list(range(NCORES))
    res = run_bass_kernel_spmd(
        nc, in_maps, core_ids=list(range(NCORES)), trace=trace, **kwargs
    )
    LAST_RESULT = res
    outs = [res.results[m]["out"] for m in range(NCORES)]
    return np.concatenate(outs, axis=1)



# revision 6
# speedup vs baseline: 1.2395x; 1.2395x over previous
"""Trainium2 Bass kernel for nn_AudioVideoInter (ragged_sequence).

Semantics (see reference): for each batch b,
  lab   = (labels[b] == 1)                       selection mask over T frames
  mean  = mean_c(video[:, b, :])                 per-frame channel mean  [T]
  vm    = compacted mean[lab]                    t selected means, in order
  scale[p] = prod_{m = max(0,p-T+t) .. min(p, t-1)} vm[m]
  out[:, b, :] = audio[:, b, :] * scale[:, None]

Key idea vs the streaming baseline: only ~t<=26 of the 1024 video frames per
batch are selected, so instead of streaming all of video (8 MiB/core) we
gather just the selected rows with two indirect DMAs (~0.5 MiB/core) and do
all scale math in the 32-slot compacted domain:
  scale[0:128]   = cumprod([vm[0:32] padded with 1, then 96 ones])  (head)
  scale[mid]     = P  (full product) for every middle 128-frame tile
  scale[T-128:T] = reverse of cumprod([vm reversed (rev-compacted), ones])
The fwd and rev compacted index lists both come from ONE gpsimd
local_scatter driven by the label ranks; the rev gather re-reads the same
rows in reverse order so both scans are plain forward cumprods in f32.

Sharding: pure data parallelism over batch. 8 cores x 4 batches each.
Within a core the 4 batches live at partitions {0,16,32,48}; the 128
gather slots (4 batches x 32 capacity, t <= 32 assumed) fill all 128
partitions, one selected video row per partition.
"""

import os
import numpy as np

T, B, C = 1024, 32, 512
NCORES = 8
BL = B // NCORES          # batches per core = 4
NT = T // 128             # 8 tiles of 128 frames
SP = 16                   # partition stride between batches
PP = BL * SP              # 64 partitions used by the per-batch pipeline
CAP = 32                  # compacted-slot capacity per batch (t <= 32)

_CACHE = {}
LAST_RESULT = None        # BassKernelResults of the most recent run (for test.py)


def _build_nc():
    import concourse.bass as bass
    import concourse.tile as tile
    from concourse import bacc, mybir
    from concourse.masks import make_identity

    f32 = mybir.dt.float32
    i32 = mybir.dt.int32
    i16 = mybir.dt.int16
    Alu = mybir.AluOpType
    Ax = mybir.AxisListType
    ActFn = mybir.ActivationFunctionType

    nc = bacc.Bacc("TRN2", target_bir_lowering=False, debug=False)

    video = nc.dram_tensor("video_feat", [T, BL, C], f32, kind="ExternalInput").ap()
    audio = nc.dram_tensor("audio_feat", [T, BL, C], f32, kind="ExternalInput").ap()
    labels = nc.dram_tensor("labels", [BL, T], i32, kind="ExternalInput").ap()
    out = nc.dram_tensor("out", [T, BL, C], f32, kind="ExternalOutput").ap()

    with tile.TileContext(nc) as tc:
        with (
            tc.tile_pool(name="inb", bufs=8) as in_pool,
            tc.tile_pool(name="outp", bufs=4) as out_pool,
            tc.tile_pool(name="small", bufs=1) as small,
            tc.tile_pool(name="psum", bufs=2, space="PSUM") as psum,
        ):
            # ---- constants / init (gpsimd, off the DVE critical path) ----
            ident = small.tile([128, 128], f32)
            make_identity(nc, ident[:])
            ones_col = small.tile([1, 128], f32)
            nc.gpsimd.memset(ones_col[:], 1.0)
            zeros = small.tile([PP, T], f32)
            nc.gpsimd.memset(zeros[:], 0.0)
            lab_i = small.tile([PP, T], i32)
            nc.gpsimd.memset(lab_i[:], 0)
            # scatter data: value 4*j at cols j and T+j (fwd and rev halves)
            j4_cat = small.tile([PP, 2 * T], i16)
            nc.gpsimd.iota(j4_cat[:], pattern=[[0, 2], [4, T]], base=0,
                           channel_multiplier=0)
            # slot iota 0..31 twice, for the r < t mask over [fwd | rev] slots
            iota64 = small.tile([PP, 2 * CAP], i16)
            nc.gpsimd.iota(iota64[:], pattern=[[0, 2], [1, CAP]], base=0,
                           channel_multiplier=0)
            # partition index -> batch offset b = p/16 (exact at rows 0,16,32,48)
            bof_i16 = small.tile([PP, 1], i16)
            nc.gpsimd.iota(bof_i16[:], pattern=[[0, 1]], base=0,
                           channel_multiplier=1)
            vmhead = small.tile([PP, 128], f32)
            nc.gpsimd.memset(vmhead[:], 1.0)
            vt2 = small.tile([PP, 128], f32)
            nc.gpsimd.memset(vt2[:], 1.0)
            mraw = small.tile([PP, 2 * CAP], f32)
            nc.gpsimd.memset(mraw[:], 0.0)

            # ---- labels -> lab mask; batch b sits at partition SP*b ----
            lab_i_spread = lab_i[:].rearrange("(b s) t -> b s t", s=SP)[:, 0, :]
            nc.sync.dma_start(out=lab_i_spread, in_=labels)

            # ---- audio stream-in (8 MiB/core; fills while indices compute) ----
            ats = []
            for t in range(NT):
                at = in_pool.tile([128, BL, C], f32, tag="inb")
                nc.sync.dma_start(out=at[:], in_=audio[t * 128 : (t + 1) * 128])
                ats.append(at)

            # ---- label pipeline: ranks -> scatter indices ----
            lab_f = small.tile([PP, T], f32)
            nc.vector.tensor_single_scalar(
                out=lab_f[:], in_=lab_i[:], scalar=1.0, op=Alu.is_equal
            )
            rank_i = small.tile([PP, T], f32)   # inclusive cumsum of lab
            nc.vector.tensor_tensor_scan(
                out=rank_i[:], data0=lab_f[:], data1=zeros[:], initial=0.0,
                op0=Alu.add, op1=Alu.add,
            )
            t_ap = rank_i[:, T - 1 : T]         # t per batch
            tm1 = small.tile([PP, 1], f32)
            nc.vector.tensor_single_scalar(
                out=tm1[:], in_=t_ap, scalar=1.0, op=Alu.subtract
            )
            tp33 = small.tile([PP, 1], f32)
            nc.vector.tensor_single_scalar(
                out=tp33[:], in_=t_ap, scalar=float(CAP + 1), op=Alu.add
            )
            # fwd: idxA = rank*lab - 1  in {-1} u [0, t-1]
            qa = small.tile([PP, T], f32)
            nc.vector.tensor_tensor(
                out=qa[:], in0=rank_i[:], in1=lab_f[:], op=Alu.mult
            )
            # rev: idxB = (t+33-rank)*lab - 1  in {-1} u [32, t+31]
            qb_pre = small.tile([PP, T], f32)
            nc.vector.tensor_scalar(
                out=qb_pre[:], in0=rank_i[:], scalar1=tp33[:], scalar2=-1.0,
                op0=Alu.subtract, op1=Alu.mult,
            )
            qb = small.tile([PP, T], f32)
            nc.vector.tensor_tensor(
                out=qb[:], in0=qb_pre[:], in1=lab_f[:], op=Alu.mult
            )
            idx_cat = small.tile([PP, 2 * T], i16)
            nc.scalar.activation(
                out=idx_cat[:, 0:T], in_=qa[:], func=ActFn.Copy, scale=1.0,
                bias=-1.0,
            )
            nc.scalar.activation(
                out=idx_cat[:, T : 2 * T], in_=qb[:], func=ActFn.Copy, scale=1.0,
                bias=-1.0,
            )

            # ---- compact the selected frame numbers (as 4*j) into slots:
            # cols 0:32 fwd order, cols 32:64 reverse order ----
            cj_cat = small.tile([PP, 2 * CAP], i16)
            nc.gpsimd.local_scatter(
                out_ap=cj_cat[:], data_ap=j4_cat[:], idxs_ap=idx_cat[:],
                channels=PP, num_elems=2 * CAP, num_idxs=2 * T,
            )
            bof_f = small.tile([PP, 1], f32)
            nc.vector.tensor_single_scalar(
                out=bof_f[:], in_=bof_i16[:], scalar=1.0 / SP, op=Alu.mult
            )
            # gather row index into flat [T*BL, C] video: 4*j + b
            cj4b = small.tile([PP, 2 * CAP], i32)
            nc.vector.tensor_scalar_add(
                out=cj4b[:], in0=cj_cat[:], scalar1=bof_f[:]
            )

            # ---- indirect gathers: one video row per partition. The HW
            # indirect-DMA path requires the canonical one-index-per-partition
            # layout, so bounce the [b, slot] indices to [128, 1] first. ----
            cj_sp = cj4b[:].rearrange(
                "(b s) (h r) -> b s h r", s=SP, h=2
            )
            idxf = small.tile([128, 1], i32)
            nc.sync.dma_start(out=idxf[:], in_=cj_sp[:, 0, 0, :])
            idxr = small.tile([128, 1], i32)
            nc.sync.dma_start(out=idxr[:], in_=cj_sp[:, 0, 1, :])
            gat = small.tile([128, 2, C], f32)
            nc.gpsimd.indirect_dma_start(
                out=gat[:, 0, :], out_offset=None,
                in_=video[:],
                in_offset=bass.IndirectOffsetOnAxis(ap=idxf[:, 0:1], axis=1),
            )
            nc.gpsimd.indirect_dma_start(
                out=gat[:, 1, :], out_offset=None,
                in_=video[:],
                in_offset=bass.IndirectOffsetOnAxis(ap=idxr[:, 0:1], axis=1),
            )

            # ---- channel sums then back to [b, slot] layout ----
            means2 = small.tile([128, 2], f32)
            nc.vector.tensor_reduce(
                out=means2[:], in_=gat[:], axis=Ax.X, op=Alu.add
            )
            mraw_sp = mraw[:].rearrange(
                "(b s) (k r) -> b s k r", s=SP, k=2
            )
            nc.sync.dma_start(out=mraw_sp[:, 0, 0, :], in_=means2[:, 0:1])
            nc.sync.dma_start(out=mraw_sp[:, 0, 1, :], in_=means2[:, 1:2])

            # ---- masked vm (slots r >= t become 1), both halves at once ----
            selm = small.tile([PP, 2 * CAP], f32)
            nc.vector.tensor_scalar(
                out=selm[:], in0=iota64[:], scalar1=tm1[:], scalar2=None,
                op0=Alu.is_le,
            )
            dm = small.tile([PP, 2 * CAP], f32)
            nc.vector.tensor_scalar(
                out=dm[:], in0=mraw[:], scalar1=1.0 / C, scalar2=-1.0,
                op0=Alu.mult, op1=Alu.add,
            )
            em = small.tile([PP, 2 * CAP], f32)
            nc.vector.tensor_tensor(
                out=em[:], in0=dm[:], in1=selm[:], op=Alu.mult
            )
            nc.vector.tensor_single_scalar(
                out=vmhead[:, 0:CAP], in_=em[:, 0:CAP], scalar=1.0, op=Alu.add
            )
            nc.vector.tensor_single_scalar(
                out=vt2[:, 0:CAP], in_=em[:, CAP : 2 * CAP], scalar=1.0,
                op=Alu.add,
            )

            # ---- compact-domain cumprods -> head/tail scale rows ----
            scale_head = small.tile([PP, 128], f32)
            nc.vector.tensor_tensor_scan(
                out=scale_head[:], data0=vmhead[:], data1=zeros[:, 0:128],
                initial=1.0, op0=Alu.mult, op1=Alu.add,
            )
            tail_ext = small.tile([PP, 128], f32)
            nc.vector.tensor_tensor_scan(
                out=tail_ext[:], data0=vt2[:], data1=zeros[:, 0:128],
                initial=1.0, op0=Alu.mult, op1=Alu.add,
            )
            tail_arr = small.tile([PP, 128], f32)
            nc.vector.tensor_copy(out=tail_arr[:, ::-1], in_=tail_ext[:])
            P_ap = scale_head[:, 127:128]

            # ---- P broadcast to [128, PP] + head/tail transposes ----
            psum_pr = psum.tile([1, PP], f32)
            nc.tensor.matmul(
                psum_pr[:], P_ap, ident[0:PP, 0:PP], start=True, stop=True
            )
            p_row = small.tile([1, PP], f32)
            nc.vector.tensor_copy(out=p_row[:], in_=psum_pr[:])
            psum_pb = psum.tile([128, PP], f32)
            nc.tensor.matmul(
                psum_pb[:], ones_col[:], p_row[:], start=True, stop=True
            )
            p_bcast = small.tile([128, PP], f32)
            nc.vector.tensor_copy(out=p_bcast[:], in_=psum_pb[:])

            sjb = small.tile([128, 2, PP], f32)
            for k, src in ((0, scale_head), (1, tail_arr)):
                pst = psum.tile([128, PP], f32)
                nc.tensor.matmul(
                    pst[:], src[:], ident[0:PP, 0:PP], start=True, stop=True
                )
                nc.vector.tensor_copy(out=sjb[:, k, :], in_=pst[:])

            # ---- output: audio tile x per-partition scale, stream out ----
            def _mult_tile(t, s_col):
                ot = out_pool.tile([128, BL, C], f32, tag="ot")
                for b in range(BL):
                    s_ap = s_col(b)
                    if b < BL // 2:
                        nc.vector.tensor_scalar_mul(
                            out=ot[:, b, :], in0=ats[t][:, b, :], scalar1=s_ap
                        )
                    else:
                        nc.scalar.mul(out=ot[:, b, :], in_=ats[t][:, b, :], mul=s_ap)
                nc.sync.dma_start(out=out[t * 128 : (t + 1) * 128], in_=ot[:])

            _mult_tile(0, lambda b: sjb[:, 0, SP * b : SP * b + 1])
            for t in range(1, NT - 1):
                _mult_tile(t, lambda b: p_bcast[:, SP * b : SP * b + 1])
            _mult_tile(NT - 1, lambda b: sjb[:, 1, SP * b : SP * b + 1])

    nc.compile()
    return nc


def _get_nc():
    if "nc" not in _CACHE:
        _CACHE["nc"] = _build_nc()
    return _CACHE["nc"]


def _ensure_ntff_hook():
    """The agent image's antenv lacks axon_hooks; provide it and register the
    ctypes-based NTFF profiling hook so trace=True works under axon."""
    import sys
    import types

    if "antenv.axon_hooks" in sys.modules:
        return
    mod = types.ModuleType("antenv.axon_hooks")
    state = {"hook": None}
    mod.set_axon_ntff_profile_hook = lambda h: state.__setitem__("hook", h)
    mod.get_axon_ntff_profile_hook = lambda: state["hook"]
    sys.modules["antenv.axon_hooks"] = mod
    try:
        from trn_agent_boot.trn_boot import _ntff_profile_via_ctypes

        so_path = "/opt/axon/libaxon_pjrt.so"
        if os.path.exists(so_path):
            mod.set_axon_ntff_profile_hook(_ntff_profile_via_ctypes(so_path))
    except Exception:
        pass


def kernel(video_feat: np.ndarray, audio_feat: np.ndarray, labels: np.ndarray) -> np.ndarray:
    global LAST_RESULT
    from concourse.bass_utils import run_bass_kernel_spmd

    video_feat = np.ascontiguousarray(video_feat, dtype=np.float32)
    audio_feat = np.ascontiguousarray(audio_feat, dtype=np.float32)
    labels = np.ascontiguousarray(labels, dtype=np.int32)

    nc = _get_nc()
    in_maps = []
    for m in range(NCORES):
        bs = slice(m * BL, (m + 1) * BL)
        in_maps.append(
            {
                "video_feat": np.ascontiguousarray(video_feat[:, bs, :]),
                "audio_feat": np.ascontiguousarray(audio_feat[:, bs, :]),
                "labels": np.ascontiguousarray(labels[bs, :]),
            }
        )

    trace = bool(os.environ.get("KERNEL_PROFILE"))
    if trace:
        _ensure_ntff_hook()
    kwargs = {}
    if trace and os.environ.get("KERNEL_PROFILE_ALL_CORES"):
        kwargs["trace_cores"] = list(range(NCORES))
    res = run_bass_kernel_spmd(
        nc, in_maps, core_ids=list(range(NCORES)), trace=trace, **kwargs
    )
    LAST_RESULT = res
    outs = [res.results[m]["out"] for m in range(NCORES)]
    return np.concatenate(outs, axis=1)


# revision 7
# speedup vs baseline: 1.3565x; 1.0944x over previous
"""Trainium2 Bass kernel for nn_AudioVideoInter (ragged_sequence).

Semantics (see reference): for each batch b,
  lab   = (labels[b] == 1)                       selection mask over T frames
  mean  = mean_c(video[:, b, :])                 per-frame channel mean  [T]
  vm    = compacted mean[lab]                    t selected means, in order
  scale[p] = prod_{m = max(0,p-T+t) .. min(p, t-1)} vm[m]
  out[:, b, :] = audio[:, b, :] * scale[:, None]

Key idea vs the streaming baseline: only ~t<=26 of the 1024 video frames per
batch are selected, so instead of streaming all of video (8 MiB/core) we
gather just the selected rows with one indirect DMA (~0.25 MiB/core) and do
all scale math in the 32-slot compacted domain (t <= 32 assumed):
  scale[0:128]   = cumprod([vm[0:32] padded with 1, then 96 ones])  (head)
  scale[mid]     = P  (full product) for every middle 128-frame tile
  scale[T-128+u] = suf[u-128+t] = prod_{m >= u-128+t} vm[m]         (tail)
The tail is built by scattering (suf[r] - P) into a zeroed row at position
128-t+r (r < t) and adding P -- a 32-index gpsimd local_scatter, so no
second gather or backward compaction is needed.

The compacted index list comes from one gpsimd local_scatter driven by the
label ranks (rank = cumsum(lab)); the selected frame numbers land in slots
0..t-1, are turned into flat video row indices (4j + b), bounced to the
canonical one-index-per-partition layout by a small SWDGE DMA, and drive an
indirect DMA gather of one video row per partition (128 slots = 4 batches x
32 capacity).

Sharding: pure data parallelism over batch. 8 cores x 4 batches each.
Within a core the 4 batches live at partitions {0,16,32,48}.
"""

import os
import numpy as np

T, B, C = 1024, 32, 512
NCORES = 8
BL = B // NCORES          # batches per core = 4
NT = T // 128             # 8 tiles of 128 frames
SP = 16                   # partition stride between batches
PP = BL * SP              # 64 partitions used by the per-batch pipeline
CAP = 32                  # compacted-slot capacity per batch (t <= 32)

_CACHE = {}
LAST_RESULT = None        # BassKernelResults of the most recent run (for test.py)


def _build_nc():
    import concourse.bass as bass
    import concourse.tile as tile
    from concourse import bacc, mybir
    from concourse.masks import make_identity

    f32 = mybir.dt.float32
    f16 = mybir.dt.float16
    i32 = mybir.dt.int32
    i16 = mybir.dt.int16
    Alu = mybir.AluOpType
    Ax = mybir.AxisListType
    ActFn = mybir.ActivationFunctionType

    nc = bacc.Bacc("TRN2", target_bir_lowering=False, debug=False)

    video = nc.dram_tensor("video_feat", [T, BL, C], f32, kind="ExternalInput").ap()
    audio = nc.dram_tensor("audio_feat", [T, BL, C], f32, kind="ExternalInput").ap()
    labels = nc.dram_tensor("labels", [BL, T], i32, kind="ExternalInput").ap()
    out = nc.dram_tensor("out", [T, BL, C], f32, kind="ExternalOutput").ap()

    with tile.TileContext(nc) as tc:
        with (
            tc.tile_pool(name="inb", bufs=8) as in_pool,
            tc.tile_pool(name="outp", bufs=4) as out_pool,
            tc.tile_pool(name="small", bufs=1) as small,
            tc.tile_pool(name="psum", bufs=2, space="PSUM") as psum,
        ):
            # ---- gpsimd preamble, ordered so the label pipeline unblocks
            # as early as possible ----
            lab_i = small.tile([PP, T], i16)
            nc.gpsimd.memset(lab_i[:], 0)
            # labels via SWDGE with i32 -> i16 cast (own descriptor rings, so
            # it is not queued behind the big HWDGE audio stream)
            lab_i_spread = lab_i[:].rearrange("(b s) t -> b s t", s=SP)[:, 0, :]
            nc.gpsimd.dma_start(out=lab_i_spread, in_=labels)
            zeros16 = small.tile([PP, T], f16)
            nc.gpsimd.memset(zeros16[:], 0.0)
            # scatter data: value 4*j at col j
            j4 = small.tile([PP, T], i16)
            nc.gpsimd.iota(j4[:], pattern=[[4, T]], base=0, channel_multiplier=0)
            # slot iota 0..31 for slot masks / tail scatter targets
            iota32 = small.tile([PP, CAP], i16)
            nc.gpsimd.iota(iota32[:], pattern=[[1, CAP]], base=0,
                           channel_multiplier=0)
            # partition index -> batch offset b = p/16 (exact at active rows)
            bof_i16 = small.tile([PP, 1], i16)
            nc.gpsimd.iota(bof_i16[:], pattern=[[0, 1]], base=0,
                           channel_multiplier=1)
            vmhead = small.tile([PP, 128], f32)
            nc.gpsimd.memset(vmhead[:], 1.0)
            mraw = small.tile([PP, CAP], f32)
            nc.gpsimd.memset(mraw[:], 0.0)
            zeros = small.tile([PP, 128], f32)
            nc.gpsimd.memset(zeros[:], 0.0)
            ident = small.tile([128, 128], f32)
            make_identity(nc, ident[:])
            ones_col = small.tile([1, 128], f32)
            nc.gpsimd.memset(ones_col[:], 1.0)

            # ---- audio stream-in (8 MiB/core; fills while indices compute) ----
            ats = []
            for t in range(NT):
                at = in_pool.tile([128, BL, C], f32, tag="inb")
                nc.sync.dma_start(out=at[:], in_=audio[t * 128 : (t + 1) * 128])
                ats.append(at)

            # ---- label pipeline (f16, 2x DVE): ranks -> scatter indices ----
            lab_f = small.tile([PP, T], f16)
            nc.vector.tensor_single_scalar(
                out=lab_f[:], in_=lab_i[:], scalar=1.0, op=Alu.is_equal
            )
            rank_i = small.tile([PP, T], f16)   # inclusive cumsum of lab
            nc.vector.tensor_tensor_scan(
                out=rank_i[:], data0=lab_f[:], data1=zeros16[:], initial=0.0,
                op0=Alu.add, op1=Alu.add,
            )
            t_ap = rank_i[:, T - 1 : T]         # t per batch (f16, exact)
            tm1 = small.tile([PP, 1], f32)
            nc.vector.tensor_single_scalar(
                out=tm1[:], in_=t_ap, scalar=1.0, op=Alu.subtract
            )
            u128mt = small.tile([PP, 1], f32)   # 128 - t
            nc.vector.tensor_scalar(
                out=u128mt[:], in0=t_ap, scalar1=-1.0, scalar2=128.0,
                op0=Alu.mult, op1=Alu.add,
            )
            # idxA = rank*lab - 1  in {-1} u [0, t-1]
            qa = small.tile([PP, T], f16)
            nc.vector.tensor_tensor(
                out=qa[:], in0=rank_i[:], in1=lab_f[:], op=Alu.mult
            )
            idxA = small.tile([PP, T], i16)
            nc.scalar.activation(
                out=idxA[:], in_=qa[:], func=ActFn.Copy, scale=1.0, bias=-1.0
            )

            # ---- compact the selected frame numbers (as 4*j) into slots ----
            cj = small.tile([PP, CAP], i16)
            nc.gpsimd.local_scatter(
                out_ap=cj[:], data_ap=j4[:], idxs_ap=idxA[:],
                channels=PP, num_elems=CAP, num_idxs=T,
            )
            bof_f = small.tile([PP, 1], f32)
            nc.vector.tensor_single_scalar(
                out=bof_f[:], in_=bof_i16[:], scalar=1.0 / SP, op=Alu.mult
            )
            # gather row index into flat [T*BL, C] video: 4*j + b
            cj4b = small.tile([PP, CAP], i32)
            nc.vector.tensor_scalar_add(
                out=cj4b[:], in0=cj[:], scalar1=bof_f[:]
            )

            # ---- indirect gather: one selected video row per partition.
            # The HW indirect-DMA path needs the canonical one-index-per-
            # partition layout, so bounce [b, slot] -> [128, 1] over SWDGE. ----
            cj_sp = cj4b[:].rearrange("(b s) r -> b s r", s=SP)
            idxf = small.tile([128, 1], i32)
            nc.gpsimd.dma_start(out=idxf[:], in_=cj_sp[:, 0, :])
            gat = small.tile([128, C], f32)
            nc.gpsimd.indirect_dma_start(
                out=gat[:], out_offset=None,
                in_=video[:],
                in_offset=bass.IndirectOffsetOnAxis(ap=idxf[:, 0:1], axis=1),
            )

            # ---- channel sums, transposed back to [b, slot] layout ----
            means1 = small.tile([128, 1], f32)
            nc.vector.tensor_reduce(
                out=means1[:], in_=gat[:], axis=Ax.X, op=Alu.add
            )
            psum_m = psum.tile([1, 128], f32)
            nc.tensor.matmul(
                psum_m[:], means1[:], ident[:], start=True, stop=True
            )
            sb1 = small.tile([1, 128], f32)
            nc.vector.tensor_copy(out=sb1[:], in_=psum_m[:])
            mraw_sp = mraw[:].rearrange("(b s) r -> b s r", s=SP)
            nc.gpsimd.dma_start(out=mraw_sp[:, 0, :], in_=sb1[:])

            # ---- masked vm (slots r >= t become 1) ----
            selm = small.tile([PP, CAP], f32)
            nc.vector.tensor_scalar(
                out=selm[:], in0=iota32[:], scalar1=tm1[:], scalar2=None,
                op0=Alu.is_le,
            )
            dm = small.tile([PP, CAP], f32)
            nc.vector.tensor_scalar(
                out=dm[:], in0=mraw[:], scalar1=1.0 / C, scalar2=-1.0,
                op0=Alu.mult, op1=Alu.add,
            )
            em = small.tile([PP, CAP], f32)
            nc.vector.tensor_tensor(
                out=em[:], in0=dm[:], in1=selm[:], op=Alu.mult
            )
            nc.vector.tensor_single_scalar(
                out=vmhead[:, 0:CAP], in_=em[:], scalar=1.0, op=Alu.add
            )
            vmh32 = small.tile([PP, CAP], f32)
            nc.vector.tensor_single_scalar(
                out=vmh32[:], in_=em[:], scalar=1.0, op=Alu.add
            )

            # ---- head scale: cumprod over [vm | ones] ----
            scale_head = small.tile([PP, 128], f32)
            nc.vector.tensor_tensor_scan(
                out=scale_head[:], data0=vmhead[:], data1=zeros[:],
                initial=1.0, op0=Alu.mult, op1=Alu.add,
            )
            P_ap = scale_head[:, 127:128]

            # ---- tail scale: suffix products suf[r] = prod_{m>=r} vm[m],
            # scattered to position 128-t+r as (suf - P), then + P ----
            suf = small.tile([PP, CAP], f32)
            nc.vector.tensor_tensor_scan(
                out=suf[:, ::-1], data0=vmh32[:, ::-1], data1=zeros[:, 0:CAP],
                initial=1.0, op0=Alu.mult, op1=Alu.add,
            )
            tdat = small.tile([PP, CAP], f16)
            nc.vector.tensor_scalar(
                out=tdat[:], in0=suf[:], scalar1=P_ap, scalar2=None,
                op0=Alu.subtract,
            )
            # target u = (iota + (128-t) + 1)*selm - 1  (slots r >= t -> -1)
            pre1 = small.tile([PP, CAP], f32)
            nc.vector.tensor_scalar(
                out=pre1[:], in0=iota32[:], scalar1=u128mt[:], scalar2=1.0,
                op0=Alu.add, op1=Alu.add,
            )
            pre2 = small.tile([PP, CAP], f32)
            nc.vector.tensor_tensor(
                out=pre2[:], in0=pre1[:], in1=selm[:], op=Alu.mult
            )
            tidx = small.tile([PP, CAP], i16)
            nc.vector.tensor_single_scalar(
                out=tidx[:], in_=pre2[:], scalar=1.0, op=Alu.subtract
            )
            dst2 = small.tile([PP, 128], f16)
            nc.gpsimd.local_scatter(
                out_ap=dst2[:], data_ap=tdat[:], idxs_ap=tidx[:],
                channels=PP, num_elems=128, num_idxs=CAP,
            )
            tail_arr = small.tile([PP, 128], f32)
            nc.vector.tensor_scalar_add(
                out=tail_arr[:], in0=dst2[:], scalar1=P_ap
            )

            # ---- P broadcast to [128, PP] + head/tail transposes ----
            psum_pr = psum.tile([1, PP], f32)
            nc.tensor.matmul(
                psum_pr[:], P_ap, ident[0:PP, 0:PP], start=True, stop=True
            )
            p_row = small.tile([1, PP], f32)
            nc.vector.tensor_copy(out=p_row[:], in_=psum_pr[:])
            psum_pb = psum.tile([128, PP], f32)
            nc.tensor.matmul(
                psum_pb[:], ones_col[:], p_row[:], start=True, stop=True
            )
            p_bcast = small.tile([128, PP], f32)
            nc.vector.tensor_copy(out=p_bcast[:], in_=psum_pb[:])

            sjb = small.tile([128, 2, PP], f32)
            for k, src in ((0, scale_head), (1, tail_arr)):
                pst = psum.tile([128, PP], f32)
                nc.tensor.matmul(
                    pst[:], src[:], ident[0:PP, 0:PP], start=True, stop=True
                )
                nc.vector.tensor_copy(out=sjb[:, k, :], in_=pst[:])

            # ---- output: audio tile x per-partition scale, stream out ----
            def _mult_tile(t, s_col):
                ot = out_pool.tile([128, BL, C], f32, tag="ot")
                for b in range(BL):
                    s_ap = s_col(b)
                    if b < BL // 2:
                        nc.vector.tensor_scalar_mul(
                            out=ot[:, b, :], in0=ats[t][:, b, :], scalar1=s_ap
                        )
                    else:
                        nc.scalar.mul(out=ot[:, b, :], in_=ats[t][:, b, :], mul=s_ap)
                nc.sync.dma_start(out=out[t * 128 : (t + 1) * 128], in_=ot[:])

            _mult_tile(0, lambda b: sjb[:, 0, SP * b : SP * b + 1])
            for t in range(1, NT - 1):
                _mult_tile(t, lambda b: p_bcast[:, SP * b : SP * b + 1])
            _mult_tile(NT - 1, lambda b: sjb[:, 1, SP * b : SP * b + 1])

    nc.compile()
    return nc


def _get_nc():
    if "nc" not in _CACHE:
        _CACHE["nc"] = _build_nc()
    return _CACHE["nc"]


def _ensure_ntff_hook():
    """The agent image's antenv lacks axon_hooks; provide it and register the
    ctypes-based NTFF profiling hook so trace=True works under axon."""
    import sys
    import types

    if "antenv.axon_hooks" in sys.modules:
        return
    mod = types.ModuleType("antenv.axon_hooks")
    state = {"hook": None}
    mod.set_axon_ntff_profile_hook = lambda h: state.__setitem__("hook", h)
    mod.get_axon_ntff_profile_hook = lambda: state["hook"]
    sys.modules["antenv.axon_hooks"] = mod
    try:
        from trn_agent_boot.trn_boot import _ntff_profile_via_ctypes

        so_path = "/opt/axon/libaxon_pjrt.so"
        if os.path.exists(so_path):
            mod.set_axon_ntff_profile_hook(_ntff_profile_via_ctypes(so_path))
    except Exception:
        pass


def kernel(video_feat: np.ndarray, audio_feat: np.ndarray, labels: np.ndarray) -> np.ndarray:
    global LAST_RESULT
    from concourse.bass_utils import run_bass_kernel_spmd

    video_feat = np.ascontiguousarray(video_feat, dtype=np.float32)
    audio_feat = np.ascontiguousarray(audio_feat, dtype=np.float32)
    labels = np.ascontiguousarray(labels, dtype=np.int32)

    nc = _get_nc()
    in_maps = []
    for m in range(NCORES):
        bs = slice(m * BL, (m + 1) * BL)
        in_maps.append(
            {
                "video_feat": np.ascontiguousarray(video_feat[:, bs, :]),
                "audio_feat": np.ascontiguousarray(audio_feat[:, bs, :]),
                "labels": np.ascontiguousarray(labels[bs, :]),
            }
        )

    trace = bool(os.environ.get("KERNEL_PROFILE"))
    if trace:
        _ensure_ntff_hook()
    kwargs = {}
    if trace and os.environ.get("KERNEL_PROFILE_ALL_CORES"):
        kwargs["trace_cores"] = list(range(NCORES))
    res = run_bass_kernel_spmd(
        nc, in_maps, core_ids=list(range(NCORES)), trace=trace, **kwargs
    )
    LAST_RESULT = res
    outs = [res.results[m]["out"] for m in range(NCORES)]
    return np.concatenate(outs, axis=1)


# revision 10
# speedup vs baseline: 1.3822x; 1.0189x over previous
"""Trainium2 Bass kernel for nn_AudioVideoInter (ragged_sequence).

Semantics (see reference): for each batch b,
  lab   = (labels[b] == 1)                       selection mask over T frames
  mean  = mean_c(video[:, b, :])                 per-frame channel mean  [T]
  vm    = compacted mean[lab]                    t selected means, in order
  scale[p] = prod_{m = max(0,p-T+t) .. min(p, t-1)} vm[m]
  out[:, b, :] = audio[:, b, :] * scale[:, None]

Only ~t<=26 of the 1024 video frames per batch are selected, so instead of
streaming all of video (8 MiB/core) we gather just the selected rows with
one indirect DMA (~0.25 MiB/core) and do all scale math in the 32-slot
compacted domain (t <= 32 assumed):
  scale[0:128]   = cumprod([vm[0:32] padded with 1, then 96 ones])  (head)
  scale[mid]     = P  (full product) for every middle 128-frame tile
  scale[T-128+u] = suf[u-128+t] = prod_{m >= u-128+t} vm[m]         (tail)
The tail is built by scattering (suf[r] - P) into a zeroed row at position
128-t+r (r < t) and adding P -- a 32-index gpsimd local_scatter.

Latency discipline (the whole scale pipeline must finish while audio still
streams): every partition-redistribution that a DMA would serialize behind
the audio stream is done on the PE instead, using constant selection
matrices:
  - slot indices [b, r] -> [128, 1]: 4 per-batch row copies into a banded
    [PP, 128] matrix, then matmul against a ones vector (column collapse).
  - gathered means [128, 1] -> [b, r]: mask a constant block-diagonal D32
    by the per-partition mean, then matmul against a batch-selection SEL.
Labels ride HWDGE first in line (before the audio chunk DMAs); audio is
fetched in 4 big 2 MiB chunks to cut HWDGE descriptor-gen serialization.

Sharding: pure data parallelism over batch. 8 cores x 4 batches each.
Within a core the 4 batches live at partitions {0,16,32,48}.
"""

import os
import numpy as np

T, B, C = 1024, 32, 512
NCORES = 8
BL = B // NCORES          # batches per core = 4
NT = T // 128             # 8 tiles of 128 frames
NCH = 4                   # audio fetched in 4 chunks of 2 tiles
SP = 32                   # partition stride between batches (32-aligned for DVE)
PP = BL * SP              # 64 partitions used by the per-batch pipeline
CAP = 32                  # compacted-slot capacity per batch (t <= 32)

_CACHE = {}
LAST_RESULT = None        # BassKernelResults of the most recent run (for test.py)


def _build_nc():
    import concourse.bass as bass
    import concourse.tile as tile
    from concourse import bacc, mybir
    from concourse.masks import make_identity

    f32 = mybir.dt.float32
    f16 = mybir.dt.float16
    i32 = mybir.dt.int32
    i16 = mybir.dt.int16
    Alu = mybir.AluOpType
    Ax = mybir.AxisListType

    nc = bacc.Bacc("TRN2", target_bir_lowering=False, debug=False)

    video = nc.dram_tensor("video_feat", [T, BL, C], f32, kind="ExternalInput").ap()
    audio = nc.dram_tensor("audio_feat", [T, BL, C], f32, kind="ExternalInput").ap()
    labels = nc.dram_tensor("labels", [BL, T], i32, kind="ExternalInput").ap()
    out = nc.dram_tensor("out", [T, BL, C], f32, kind="ExternalOutput").ap()

    with tile.TileContext(nc) as tc:
        with (
            tc.tile_pool(name="inb", bufs=NCH) as in_pool,
            tc.tile_pool(name="outp", bufs=4) as out_pool,
            tc.tile_pool(name="small", bufs=1) as small,
            tc.tile_pool(name="psum", bufs=2, space="PSUM") as psum,
        ):
            # ---- tiles the label DMA / pipeline needs first; memset on DVE
            # (free at startup) so the HWDGE labels DMA is not gated on the
            # gpsimd preamble ----
            lab_i = small.tile([PP, T], i32)
            nc.vector.memset(lab_i[:], 0)
            zeros16 = small.tile([PP, T], f16)
            nc.vector.memset(zeros16[:], 0.0)

            # ---- labels first in the HWDGE line, before the audio chunks ----
            lab_i_spread = lab_i[:].rearrange("(b s) t -> b s t", s=SP)[:, 0, :]
            nc.sync.dma_start(out=lab_i_spread, in_=labels)

            # ---- audio stream-in: 4 chunks x 2 MiB ----
            chunks = []
            for c in range(NCH):
                ch = in_pool.tile([128, 2, BL, C], f32, tag="inb")
                src = audio[256 * c : 256 * (c + 1)].rearrange(
                    "(k p) b c -> p k b c", p=128
                )
                nc.sync.dma_start(out=ch[:], in_=src)
                chunks.append(ch)

            def audio_tile(t):
                return chunks[t // 2][:, t % 2, :, :]

            # ---- gpsimd preamble (constants; all off the critical path) ----
            j4 = small.tile([PP, T], i16)
            nc.gpsimd.iota(j4[:], pattern=[[4, T]], base=0, channel_multiplier=0)
            iota32 = small.tile([PP, CAP], i16)
            nc.gpsimd.iota(iota32[:], pattern=[[1, CAP]], base=0,
                           channel_multiplier=0)
            md2 = small.tile([PP, 128], f32)
            nc.gpsimd.memset(md2[:], 0.0)
            ones_pp = small.tile([PP, 1], f32)
            nc.gpsimd.memset(ones_pp[:], 1.0)
            vmhead = small.tile([PP, 128], f32)
            nc.gpsimd.memset(vmhead[:], 1.0)
            zeros = small.tile([PP, 128], f32)
            nc.gpsimd.memset(zeros[:], 0.0)
            # D32[q, r] = (q % 32 == r): block diagonal
            d32 = small.tile([128, CAP], f32)
            nc.gpsimd.memset(d32[:], 0.0)
            for j in range(4):
                nc.gpsimd.affine_select(
                    out=d32[32 * j : 32 * (j + 1), :],
                    in_=d32[32 * j : 32 * (j + 1), :],
                    compare_op=Alu.not_equal, fill=1.0, base=0,
                    pattern=[[-1, CAP]], channel_multiplier=1,
                )
            # SEL[q, f] = 1 iff f = SP*(q//32): batch collector
            sel = small.tile([128, PP], f32)
            nc.gpsimd.memset(sel[:], 0.0)
            for b in range(BL):
                nc.gpsimd.memset(sel[32 * b : 32 * (b + 1), SP * b : SP * b + 1], 1.0)
            ident = small.tile([128, 128], f32)
            make_identity(nc, ident[:])
            ones_col = small.tile([1, 128], f32)
            nc.gpsimd.memset(ones_col[:], 1.0)

            # ---- label pipeline (f16): ranks -> compaction indices ----
            lab_f = small.tile([PP, T], f16)
            nc.vector.tensor_single_scalar(
                out=lab_f[:], in_=lab_i[:], scalar=1.0, op=Alu.is_equal
            )
            rank_i = small.tile([PP, T], f16)   # inclusive cumsum of lab
            nc.vector.tensor_tensor_scan(
                out=rank_i[:], data0=lab_f[:], data1=zeros16[:], initial=0.0,
                op0=Alu.add, op1=Alu.add,
            )
            t_ap = rank_i[:, T - 1 : T]         # t per batch (f16, exact)
            tm1 = small.tile([PP, 1], f32)
            nc.vector.tensor_single_scalar(
                out=tm1[:], in_=t_ap, scalar=1.0, op=Alu.subtract
            )
            u128mt = small.tile([PP, 1], f32)   # 128 - t
            nc.vector.tensor_scalar(
                out=u128mt[:], in0=t_ap, scalar1=-1.0, scalar2=128.0,
                op0=Alu.mult, op1=Alu.add,
            )
            # idxA = rank*lab - 1  in {-1} u [0, t-1]
            qa = small.tile([PP, T], f16)
            nc.vector.tensor_tensor(
                out=qa[:], in0=rank_i[:], in1=lab_f[:], op=Alu.mult
            )
            idxA = small.tile([PP, T], i16)
            nc.vector.tensor_single_scalar(
                out=idxA[:], in_=qa[:], scalar=1.0, op=Alu.subtract
            )

            # ---- compact the selected frame numbers (as 4*j) into slots ----
            cj = small.tile([PP, CAP], i16)
            nc.gpsimd.local_scatter(
                out_ap=cj[:], data_ap=j4[:], idxs_ap=idxA[:],
                channels=PP, num_elems=CAP, num_idxs=T,
            )
            # gather row index into flat [T*BL, C] video: 4*j + b, as f32 so
            # the PE can redistribute it to the one-index-per-partition layout
            cj4b = small.tile([PP, CAP], f32)
            for b in range(BL):
                nc.vector.tensor_single_scalar(
                    out=cj4b[SP * b : SP * b + 1, :],
                    in_=cj[SP * b : SP * b + 1, :], scalar=float(b), op=Alu.add,
                )
            # banded copy: md2[16b, 32b + r] = cj4b[16b, r]; column-collapse
            # via ones matmul puts slot q's index at psum partition q
            for b in range(BL):
                nc.vector.tensor_copy(
                    out=md2[SP * b : SP * b + 1, 32 * b : 32 * (b + 1)],
                    in_=cj4b[SP * b : SP * b + 1, :],
                )
            psum_idx = psum.tile([128, 1], f32, tag="ps")
            nc.tensor.matmul(
                psum_idx[:], md2[:], ones_pp[:], start=True, stop=True
            )
            idxf = small.tile([128, 1], i32)
            nc.vector.tensor_copy(out=idxf[:], in_=psum_idx[:])

            # ---- indirect gather: one selected video row per partition ----
            gat = small.tile([128, C], f32)
            nc.gpsimd.indirect_dma_start(
                out=gat[:], out_offset=None,
                in_=video[:],
                in_offset=bass.IndirectOffsetOnAxis(ap=idxf[:, 0:1], axis=1),
            )

            # ---- channel sums; PE-redistribute to [b, slot] layout:
            # mraw[SPb, r] = sum_q SEL[q, SPb] * (means1[q] * D32[q, r]) ----
            means1 = small.tile([128, 1], f32)
            nc.vector.tensor_reduce(
                out=means1[:], in_=gat[:], axis=Ax.X, op=Alu.add
            )
            md = small.tile([128, CAP], f32)
            nc.vector.tensor_scalar_mul(
                out=md[:], in0=d32[:], scalar1=means1[:, 0:1]
            )
            psum_mr = psum.tile([PP, CAP], f32, tag="ps")
            nc.tensor.matmul(
                psum_mr[:], sel[:], md[:], start=True, stop=True
            )
            mraw = small.tile([PP, CAP], f32)
            nc.vector.tensor_copy(out=mraw[:], in_=psum_mr[:])

            # ---- masked vm (slots r >= t become 1) ----
            selm = small.tile([PP, CAP], f32)
            nc.vector.tensor_scalar(
                out=selm[:], in0=iota32[:], scalar1=tm1[:], scalar2=None,
                op0=Alu.is_le,
            )
            # tail scatter targets u = (iota + (128-t) + 1)*selm - 1
            # (independent of the means; compute early)
            pre1 = small.tile([PP, CAP], f32)
            nc.vector.tensor_scalar(
                out=pre1[:], in0=iota32[:], scalar1=u128mt[:], scalar2=1.0,
                op0=Alu.add, op1=Alu.add,
            )
            pre2 = small.tile([PP, CAP], f32)
            nc.vector.tensor_tensor(
                out=pre2[:], in0=pre1[:], in1=selm[:], op=Alu.mult
            )
            tidx = small.tile([PP, CAP], i16)
            nc.vector.tensor_single_scalar(
                out=tidx[:], in_=pre2[:], scalar=1.0, op=Alu.subtract
            )
            dm = small.tile([PP, CAP], f32)
            nc.vector.tensor_scalar(
                out=dm[:], in0=mraw[:], scalar1=1.0 / C, scalar2=-1.0,
                op0=Alu.mult, op1=Alu.add,
            )
            em = small.tile([PP, CAP], f32)
            nc.vector.tensor_tensor(
                out=em[:], in0=dm[:], in1=selm[:], op=Alu.mult
            )
            nc.vector.tensor_single_scalar(
                out=vmhead[:, 0:CAP], in_=em[:], scalar=1.0, op=Alu.add
            )
            vmh32 = small.tile([PP, CAP], f32)
            nc.vector.tensor_single_scalar(
                out=vmh32[:], in_=em[:], scalar=1.0, op=Alu.add
            )

            # ---- head scale: cumprod over [vm | ones] ----
            scale_head = small.tile([PP, 128], f32)
            nc.vector.tensor_tensor_scan(
                out=scale_head[:], data0=vmhead[:], data1=zeros[:],
                initial=1.0, op0=Alu.mult, op1=Alu.add,
            )
            P_ap = scale_head[:, 127:128]

            # ---- tail scale: suffix products scattered as (suf - P), + P ----
            suf = small.tile([PP, CAP], f32)
            nc.vector.tensor_tensor_scan(
                out=suf[:, ::-1], data0=vmh32[:, ::-1], data1=zeros[:, 0:CAP],
                initial=1.0, op0=Alu.mult, op1=Alu.add,
            )
            tdat = small.tile([PP, CAP], f16)
            nc.vector.tensor_scalar(
                out=tdat[:], in0=suf[:], scalar1=P_ap, scalar2=None,
                op0=Alu.subtract,
            )
            dst2 = small.tile([PP, 128], f16)
            nc.gpsimd.local_scatter(
                out_ap=dst2[:], data_ap=tdat[:], idxs_ap=tidx[:],
                channels=PP, num_elems=128, num_idxs=CAP,
            )
            tail_arr = small.tile([PP, 128], f32)
            nc.vector.tensor_scalar_add(
                out=tail_arr[:], in0=dst2[:], scalar1=P_ap
            )

            # ---- P broadcast to [128, PP] + head/tail transposes ----
            psum_pr = psum.tile([1, PP], f32, tag="ps")
            nc.tensor.matmul(
                psum_pr[:], P_ap, ident[0:PP, 0:PP], start=True, stop=True
            )
            p_row = small.tile([1, PP], f32)
            nc.vector.tensor_copy(out=p_row[:], in_=psum_pr[:])
            psum_pb = psum.tile([128, PP], f32, tag="ps")
            nc.tensor.matmul(
                psum_pb[:], ones_col[:], p_row[:], start=True, stop=True
            )
            p_bcast = small.tile([128, PP], f32)
            nc.vector.tensor_copy(out=p_bcast[:], in_=psum_pb[:])

            sjb = small.tile([128, 2, PP], f32)
            for k, src in ((0, scale_head), (1, tail_arr)):
                pst = psum.tile([128, PP], f32, tag="ps")
                nc.tensor.matmul(
                    pst[:], src[:], ident[0:PP, 0:PP], start=True, stop=True
                )
                nc.vector.tensor_copy(out=sjb[:, k, :], in_=pst[:])

            # ---- output: audio tile x per-partition scale, stream out ----
            def _mult_tile(t, s_col):
                ot = out_pool.tile([128, BL, C], f32, tag="ot")
                at = audio_tile(t)
                for b in range(BL):
                    s_ap = s_col(b)
                    if b < 3:
                        nc.vector.tensor_scalar_mul(
                            out=ot[:, b, :], in0=at[:, b, :], scalar1=s_ap
                        )
                    else:
                        nc.scalar.mul(out=ot[:, b, :], in_=at[:, b, :], mul=s_ap)
                nc.sync.dma_start(out=out[t * 128 : (t + 1) * 128], in_=ot[:])

            _mult_tile(0, lambda b: sjb[:, 0, SP * b : SP * b + 1])
            for t in range(1, NT - 1):
                _mult_tile(t, lambda b: p_bcast[:, SP * b : SP * b + 1])
            _mult_tile(NT - 1, lambda b: sjb[:, 1, SP * b : SP * b + 1])

    nc.compile()
    return nc


def _get_nc():
    if "nc" not in _CACHE:
        _CACHE["nc"] = _build_nc()
    return _CACHE["nc"]


def _ensure_ntff_hook():
    """The agent image's antenv lacks axon_hooks; provide it and register the
    ctypes-based NTFF profiling hook so trace=True works under axon."""
    import sys
    import types

    if "antenv.axon_hooks" in sys.modules:
        return
    mod = types.ModuleType("antenv.axon_hooks")
    state = {"hook": None}
    mod.set_axon_ntff_profile_hook = lambda h: state.__setitem__("hook", h)
    mod.get_axon_ntff_profile_hook = lambda: state["hook"]
    sys.modules["antenv.axon_hooks"] = mod
    try:
        from trn_agent_boot.trn_boot import _ntff_profile_via_ctypes

        so_path = "/opt/axon/libaxon_pjrt.so"
        if os.path.exists(so_path):
            mod.set_axon_ntff_profile_hook(_ntff_profile_via_ctypes(so_path))
    except Exception:
        pass


def kernel(video_feat: np.ndarray, audio_feat: np.ndarray, labels: np.ndarray) -> np.ndarray:
    global LAST_RESULT
    from concourse.bass_utils import run_bass_kernel_spmd

    video_feat = np.ascontiguousarray(video_feat, dtype=np.float32)
    audio_feat = np.ascontiguousarray(audio_feat, dtype=np.float32)
    labels = np.ascontiguousarray(labels, dtype=np.int32)

    nc = _get_nc()
    in_maps = []
    for m in range(NCORES):
        bs = slice(m * BL, (m + 1) * BL)
        in_maps.append(
            {
                "video_feat": np.ascontiguousarray(video_feat[:, bs, :]),
                "audio_feat": np.ascontiguousarray(audio_feat[:, bs, :]),
                "labels": np.ascontiguousarray(labels[bs, :]),
            }
        )

    trace = bool(os.environ.get("KERNEL_PROFILE"))
    if trace:
        _ensure_ntff_hook()
    kwargs = {}
    if trace and os.environ.get("KERNEL_PROFILE_ALL_CORES"):
        kwargs["trace_cores"] = list(range(NCORES))
    res = run_bass_kernel_spmd(
        nc, in_maps, core_ids=list(range(NCORES)), trace=trace, **kwargs
    )
    LAST_RESULT = res
    outs = [res.results[m]["out"] for m in range(NCORES)]
    return np.concatenate(outs, axis=1)
